# revision 33
# baseline (speedup 1.0000x reference)
"""Trainium2 Bass kernel for nn_Attention (B=8, L=2048, D=64).

Reference (per batch b):
    BZ = x @ B_w.T + B_b
    CZ = x @ C_w.T + C_b
    scores = BZ @ CZ.T              # (L, L)
    attn = relu(scores)
    attn = attn / (attn.sum(axis=-2, keepdims=True) + EPS)   # column-sum norm
    VZ = x @ V_w.T + V_b
    out = x + attn @ VZ

Strategy (one batch per NeuronCore, 8 cores, no cross-core comms):
  * Host pre-transposes x into x_aug^T = [x.T; 1] (65 x 2048, fp16) so the
    kernel never runs PE transposes; biases fold via the augmented row.
  * Projections BZ^T / CZ^T computed into BOTH partition halves directly
    (PE column tiling h0/h64) so the K=64 scores matmuls can be
    row-packed: two m-chunks run concurrently in PE row groups.
  * S^T orientation puts the column-normalization axis on the free dim;
    relu + column-sums fuse into the PSUM->SBUF evacuation
    (ACT activation accum_out / DVE tensor_scalar accum_out; chunk A is
    ACT-owned, chunk B DVE-owned).  This evacuation is the hard
    throughput floor: ACT+DVE are the only engines with PSUM ports,
    1 elem/cycle/lane.
  * 3-stage software pipeline per chunk pair p:
      scores/evac(p) | den-merge(p) on GpSimd | recip+VZ-scale(p-1) on
      DVE | O-matmuls(p-2)
    so the strict-FIFO DVE never waits on the GpSimd round trip and the
    in-order PE never waits on the relu/normalize chain.  u1 scores are
    issued right after the u0 evacuation frees their PSUM buffer so the
    next pair is never gated on a late evacuation.
  * Normalization folds into VZ rows: O^T = (VZ*recip)^T @ A^T,
    accumulated in PSUM over m-chunks, column-packed into [128, 1024]
    (l lower half on partitions 0-63, upper on 64-127).
  * The +x residual is 4 identity matmuls accumulated into the same
    PSUM ahead of the O matmuls (start=True); the host un-packs/
    transposes the [128,1024] result, so the epilogue is just a
    4-quarter PSUM evac + DMA out.
  * Dummy filler matmuls into a dedicated PSUM bank plug the remaining
    PE gaps, holding the HAM clock gate at 2.4 GHz for the whole kernel
    (any ~1us PE-idle window re-throttles the PE to 1.2 GHz).
  * All PE matmuls run in fp16 (fp32 PSUM accumulation).
"""

import os
import sys

sys.path.insert(0, "/opt/trn_rl_repo")

import numpy as np

import concourse.bacc as bacc
import concourse.tile as tile
from concourse import mybir
from concourse import bass_utils

L = 2048
D = 64
DA = D + 1          # augmented feature dim
P = 128
NCH = L // P        # 16 m-chunks
SL = 512            # matmul moving-slice width
NSL = L // SL       # 4 slices
EU = 1024           # relu-evacuation unit width (2 PSUM banks)
EPS = 1e-8
N_CORES = 8

F32 = mybir.dt.float32
F16 = mybir.dt.float16


def _attention_kernel(ctx, tc, yt_ap, xat_ap, w_ap, cfg):
    nc = tc.nc
    Relu = mybir.ActivationFunctionType.Relu
    Copy = mybir.ActivationFunctionType.Copy
    Alu = mybir.AluOpType
    at_dt = F16

    consts = ctx.enter_context(tc.tile_pool(name="consts", bufs=1))
    bigs = ctx.enter_context(tc.tile_pool(name="bigs", bufs=1))
    at_pool = ctx.enter_context(tc.tile_pool(name="at", bufs=7))
    small = ctx.enter_context(tc.tile_pool(name="small", bufs=8))

    # one packed weight DMA: [B_aug | C_aug | V_aug | ident]
    w_sb = consts.tile([DA, 4 * D], F16)
    nc.scalar.dma_start(out=w_sb, in_=w_ap)
    b_sb = w_sb[:, 0:D]
    c_sb = w_sb[:, D : 2 * D]
    v_sb = w_sb[:, 2 * D : 3 * D]
    ident = w_sb[0:D, 3 * D : 4 * D]

    # x_aug^T, host-prepared: one half per HWDGE queue
    xT = bigs.tile([DA, L], F16)
    nc.sync.dma_start(out=xT[:, 0:EU], in_=xat_ap[:, 0:EU])
    nc.scalar.dma_start(out=xT[:, EU : 2 * EU], in_=xat_ap[:, EU : 2 * EU])

    bz = bigs.tile([P, L], F16)           # BZ^T duplicated on both halves
    cz = bigs.tile([P, L], F16)           # CZ^T duplicated on both halves
    vz_sb = bigs.tile([P, NCH, D], F32)   # VZ natural
    yt_sb = bigs.tile([P, EU], F32)       # O^T + x^T col-packed staging

    wu_a = consts.tile([P, SL], F16)
    nc.vector.memset(wu_a, 0.25)
    wu_res = consts.tile([P, 1], F32)

    # O^T accumulator lives for the whole kernel: 2 PSUM banks
    po_pool = ctx.enter_context(tc.tile_pool(name="po", bufs=1, space="PSUM"))
    po = po_pool.tile([P, EU], F32)
    # shared warmup/filler bank (dummy matmuls keep the HAM gate open)
    fil_pool = ctx.enter_context(tc.tile_pool(name="fil", bufs=1, space="PSUM"))
    fil = fil_pool.tile([P, SL], F32)

    def filler(n, rhs=None):
        # Dummy matmuls that keep the PE busy.  When `rhs` is given the
        # filler DEPENDS on in-flight data, pinning it to its pipeline
        # slot — the static scheduler would otherwise hoist dep-free
        # work to the front of the program and delay real matmuls.
        for _ in range(n):
            nc.tensor.matmul(fil, wu_a[:, 0:P],
                             wu_a if rhs is None else rhs,
                             start=True, stop=True, skip_group_check=True)

    # ---------------- prologue ----------------
    # Warmup burst bridges the input-DMA wait (trips the HAM clock gate);
    # the projections follow immediately.
    filler(cfg["warmup"])
    nc.vector.tensor_copy(wu_res, fil[:, 0:1])

    pv_pool = ctx.enter_context(tc.tile_pool(name="pv", bufs=1, space="PSUM"))
    with tc.tile_pool(name="pp", bufs=2, space="PSUM") as pp_pool:
        def proj_half(w_t, dst, u):
            pp = pp_pool.tile([P, EU], F32, tag="pp")
            for jj in range(2):
                j = 2 * u + jj
                sl = slice(SL * jj, SL * (jj + 1))
                xs = xT[:, SL * j : SL * (j + 1)]
                nc.tensor.matmul(pp[0:D, sl], w_t, xs, start=True, stop=True)
                nc.tensor.matmul(pp[D : 2 * D, sl], w_t, xs,
                                 start=True, stop=True)
            if u == 0:
                nc.scalar.activation(out=dst[:, 0:EU], in_=pp, func=Copy)
            else:
                nc.vector.tensor_copy(dst[:, EU : 2 * EU], pp)

        proj_half(b_sb, bz, 0)      # needs x[:, 0:1024]     -> ACT evac
        proj_half(c_sb, cz, 0)      # needs x[:, 0:1024]     -> ACT evac
        proj_half(b_sb, bz, 1)      # needs x[:, 1024:2048]  -> DVE evac
        proj_half(c_sb, cz, 1)      # -> DVE evac

    # VZ and the +x residual are emitted INSIDE the early main loop as
    # real PE filler; deferred emitters:
    def vz_group(g):
        pv = pv_pool.tile([P, 8, D], F32, tag="pv")
        for j in range(8):
            c = 8 * g + j
            nc.tensor.matmul(pv[:, j, :], xT[:, P * c : P * (c + 1)],
                             v_sb, start=True, stop=True)
        nc.scalar.activation(out=vz_sb[:, 8 * g : 8 * (g + 1), :],
                             in_=pv, func=Copy)

    def residual():
        # identity matmuls seed the O^T accumulator (must precede emit_o)
        for j in range(4):
            if j < 2:
                out_ap = po[0:D, SL * j : SL * (j + 1)]
            else:
                out_ap = po[D : 2 * D, SL * (j - 2) : SL * (j - 1)]
            nc.tensor.matmul(out_ap, ident, xT[0:D, SL * j : SL * (j + 1)],
                             start=True, stop=False)

    # ---------------- main loop ----------------
    def emit_scores(pstile, lo, cc, u):
        for jj in range(2):
            j = 2 * u + jj
            nc.tensor.matmul(pstile[:, SL * jj : SL * (jj + 1)],
                             cz[lo : lo + D, P * cc : P * (cc + 1)],
                             bz[lo : lo + D, SL * j : SL * (j + 1)],
                             start=True, stop=True)

    def emit_chain_tail(ch):
        # recip (DVE) + VZ row scaling for a finished pair.  The scaling
        # runs on GpSimd — slow per element but entirely off the two
        # PSUM-evacuation engines, and the O matmuls consuming it trail
        # by two pair-slots so the latency is hidden.
        cA, csAB = ch[0], ch[2]
        recip = small.tile([P, 2], F32, tag="recip")
        nc.vector.reciprocal(recip, csAB)
        vzs2 = small.tile([P, 2, D], at_dt, tag="vzs")
        if cfg["gps_scale"]:
            nc.gpsimd.tensor_scalar(out=vzs2[:, 0, :], in0=vz_sb[:, cA, :],
                                    scalar1=recip[:, 0:1], scalar2=None,
                                    op0=Alu.mult)
            nc.gpsimd.tensor_scalar(out=vzs2[:, 1, :], in0=vz_sb[:, cA + 1, :],
                                    scalar1=recip[:, 1:2], scalar2=None,
                                    op0=Alu.mult)
        else:
            nc.vector.tensor_tensor(
                out=vzs2, in0=vz_sb[:, cA : cA + 2, :],
                in1=recip.unsqueeze(2).broadcast_to([P, 2, D]),
                op=Alu.mult)
        return vzs2

    def emit_o(c, at, vzs):
        # column-packed: j 0/1 -> partitions 0-63, j 2/3 -> 64-127
        for j in range(4):
            if j < 2:
                out_ap = po[0:D, SL * j : SL * (j + 1)]
            else:
                out_ap = po[D : 2 * D, SL * (j - 2) : SL * (j - 1)]
            nc.tensor.matmul(out_ap, vzs, at[:, SL * j : SL * (j + 1)],
                             start=False, stop=(c == NCH - 1))

    with tc.tile_pool(name="ps", bufs=2, space="PSUM") as ps_pool:
        pend_chain = None     # (cA, cB, csAB, (atA, atB)) awaiting recip
        pend_o = []           # ((cA, (atA, atB), vzs2)) awaiting O matmuls
        for p in range(NCH // 2):
            cA, cB = 2 * p, 2 * p + 1
            atA = at_pool.tile([P, L], at_dt, tag="at")
            atB = at_pool.tile([P, L], at_dt, tag="at")
            cs2A = small.tile([P, 2], F32, tag="cs2A")
            cs2B = small.tile([P, 2], F32, tag="cs2B")
            # ---- u0 halves: row-packed scores + evacuation
            psA0 = ps_pool.tile([P, EU], F32, tag="ps")
            psB0 = ps_pool.tile([P, EU], F32, tag="ps")
            emit_scores(psA0, 0, cA, 0)
            emit_scores(psB0, D, cB, 0)
            nc.scalar.activation(out=atA[:, 0:EU], in_=psA0,
                                 func=Relu, accum_out=cs2A[:, 0:1])
            nc.vector.tensor_scalar(out=atB[:, 0:EU], in0=psB0,
                                    scalar1=0.0, scalar2=EPS,
                                    op0=Alu.max, op1=Alu.add,
                                    accum_out=cs2B[:, 0:1])
            if p == 0:
                vz_group(0)       # real PE work woven into the early loop
            elif p == 1:
                vz_group(1)
            elif p >= 2:
                filler(cfg["fill0"], rhs=prev_atA[:, 0:SL])
            # ---- u1 halves right after the u0 evacs free their buffers
            psA1 = ps_pool.tile([P, EU], F32, tag="ps")
            psB1 = ps_pool.tile([P, EU], F32, tag="ps")
            emit_scores(psA1, 0, cA, 1)
            emit_scores(psB1, D, cB, 1)
            nc.scalar.activation(out=atA[:, EU : 2 * EU], in_=psA1,
                                 func=Relu, accum_out=cs2A[:, 1:2])
            nc.vector.tensor_scalar(out=atB[:, EU : 2 * EU], in0=psB1,
                                    scalar1=0.0, scalar2=EPS,
                                    op0=Alu.max, op1=Alu.add,
                                    accum_out=cs2B[:, 1:2])
            # ---- chain tail for the previous pair + O two pairs back
            if pend_chain is not None:
                vzs2 = emit_chain_tail(pend_chain)
                pend_o.append((pend_chain[0], pend_chain[3], vzs2))
            if p == 1:
                residual()        # po seeded before the first emit_o
            if len(pend_o) >= 2:
                c0, ats, v0 = pend_o.pop(0)
                emit_o(c0, ats[0], v0[:, 0, :])
                emit_o(c0 + 1, ats[1], v0[:, 1, :])
            if p >= 2:
                filler(cfg["fill1"], rhs=prev_atA[:, EU : EU + SL])
            prev_atA = atA
            # ---- den merge on GpSimd (hidden behind next pair's evac)
            csAB = small.tile([P, 2], F32, tag="csAB")
            meng = nc.gpsimd if cfg["gps_merge"] else nc.vector
            meng.tensor_tensor(out=csAB[:, 0:1], in0=cs2A[:, 0:1],
                               in1=cs2A[:, 1:2], op=Alu.add)
            meng.tensor_tensor(out=csAB[:, 1:2], in0=cs2B[:, 0:1],
                               in1=cs2B[:, 1:2], op=Alu.add)
            pend_chain = (cA, cB, csAB, (atA, atB))
        # ---- drain the pipeline: O for the second-to-last pair first
        # so the PE is busy while the last pair's chain tail resolves
        c0, ats, v0 = pend_o.pop(0)
        emit_o(c0, ats[0], v0[:, 0, :])
        emit_o(c0 + 1, ats[1], v0[:, 1, :])
        vzs2 = emit_chain_tail(pend_chain)
        c0, ats = pend_chain[0], pend_chain[3]
        emit_o(c0, ats[0], vzs2[:, 0, :])
        emit_o(c0 + 1, ats[1], vzs2[:, 1, :])

    # ---------------- epilogue ----------------
    # quarter-granularity PSUM evac so each output DMA starts early
    Q = EU // 4
    for q in range(4):
        sl = slice(Q * q, Q * (q + 1))
        if q < 2:
            nc.scalar.activation(out=yt_sb[:, sl], in_=po[:, sl], func=Copy)
        else:
            nc.vector.tensor_copy(yt_sb[:, sl], po[:, sl])
        dma = (nc.sync, nc.scalar, nc.sync, nc.scalar)[q]
        dma.dma_start(out=yt_ap[:, sl], in_=yt_sb[:, sl])


_CACHE = {}


def _build(gps_merge=True, warmup=8, fill0=1, fill1=1, gps_scale=True):
    key = ("nc", gps_merge, warmup, fill0, fill1, gps_scale)
    if key in _CACHE:
        return _CACHE[key]
    cfg = {"gps_merge": gps_merge, "warmup": warmup,
           "fill0": fill0, "fill1": fill1, "gps_scale": gps_scale}
    nc = bacc.Bacc("TRN2", target_bir_lowering=False, debug=False,
                   enable_asserts=False, num_devices=1)
    xat = nc.dram_tensor("xat", (DA, L), F16, kind="ExternalInput").ap()
    w = nc.dram_tensor("wpack", (DA, 4 * D), F16, kind="ExternalInput").ap()
    yt = nc.dram_tensor("yt", (P, EU), F32, kind="ExternalOutput").ap()
    from contextlib import ExitStack
    with tile.TileContext(nc) as tc, ExitStack() as ctx:
        _attention_kernel(ctx, tc, yt, xat, w, cfg)
    nc.compile()
    _CACHE[key] = nc
    return nc


def _fold_weights(B_w, B_b, C_w, C_b, V_w, V_b):
    def aug(w, bias):
        full = np.concatenate(
            [np.asarray(w, np.float32).T, np.asarray(bias, np.float32)[None, :]],
            axis=0)
        return full.astype(np.float16)
    return aug(B_w, B_b), aug(C_w, C_b), aug(V_w, V_b)


def run(inputs, trace=False, tmpdir=None, gps_merge=True, warmup=8,
        fill0=1, fill1=1, gps_scale=True):
    nc = _build(gps_merge, warmup, fill0, fill1, gps_scale)
    x = np.asarray(inputs["x"], dtype=np.float32)
    b_augt, c_augt, v_augt = _fold_weights(
        inputs["B_w"], inputs["B_b"], inputs["C_w"], inputs["C_b"],
        inputs["V_w"], inputs["V_b"])
    ident = np.concatenate(
        [np.eye(D, dtype=np.float16), np.zeros((1, D), np.float16)], axis=0)
    wpack = np.concatenate([b_augt, c_augt, v_augt, ident], axis=1)
    ones = np.ones((1, L), np.float16)
    in_maps = []
    for i in range(N_CORES):
        xat = np.concatenate(
            [np.ascontiguousarray(x[i].T).astype(np.float16), ones], axis=0)
        in_maps.append({"xat": xat, "wpack": wpack})
    res = bass_utils.run_bass_kernel_spmd(nc, in_maps,
                                          core_ids=list(range(N_CORES)),
                                          trace=trace, tmpdir=tmpdir)
    out = np.empty((N_CORES, L, D), np.float32)
    for i in range(N_CORES):
        yt = res.results[i]["yt"]
        out[i, 0:EU, :] = yt[0:D, :].T
        out[i, EU : 2 * EU, :] = yt[D : 2 * D, :].T
    return out, res


def kernel(**inputs) -> np.ndarray:
    out, _ = run(inputs, trace=False)
    return out
